# revision 22
# baseline (speedup 1.0000x reference)
"""Trainium2 Bass kernel for nn_NewAttention (sparse gaussian attention).

Reference computation (B=8, L=1024, E=1024, H=8, d=128):
    v    = (values @ Wi.T)                        # [B, L, E] per-position projection
    v    = where(key_mask, 0, v)                  # zero masked key rows
    att  = einsum('hqv,bhvd->bhqd', gauss, v_h)   # per-head gaussian positional conv
    out  = att_merged @ Wo.T                      # [B, L, E]

The gaussian weights w[h,q,v] = pdf(v - ofs_h - q; std=1) form a narrow band,
so the attention is a per-head 1-D convolution along the sequence; all heads
share the same centered gaussian, the per-head integer offset only shifts
where the result is read.  Per core (data-parallel over B, one batch per
NeuronCore, no collectives):

  mm1:  vp_T[c, v] = WiT[e, c].T @ AT[e, v]     (PE; AT = values[b].T host-prepped)
  conv: U[c, j] = sum_k g(k) * vp_T[c, j + k], j in [-3, 1027), k in [-5, 5]
        -- 11 fused multiply-adds per head block on the otherwise-idle VectorE.
  mm2:  out[q, m] = U[c, q + ofs_h].T @ WoT[c, m] (PE; per-head shift is just a
        free-dim offset into U).

Matmul precision: the PE's native fp32 mode costs 4 cycles/column (2 LOW_HIGH
passes).  Instead each product X @ W is computed in 3 single-cycle passes:
    X_hi @ W_hi  (float32r: fp32 bits rounded to 11 mantissa bits on read)
  + bf16(X)  @ bf16(W - W_hi)
  + bf16(X - X_hi) @ bf16(W)
where X_hi = round-to-nearest-11-bits(X) so the f32r read is exact.  The hi*hi
products are exact in fp32 accumulation; cross terms are ~2^-12 corrections
with ~2^-8 relative error of their own -> total ~fp32 accuracy at ~3/4 the PE
cost.  Weight splits are host-side; the U split runs on VectorE/ScalarE.
PSUM accumulates everything in fp32.
"""

import math

import numpy as np
import ml_dtypes

import concourse.bacc as bacc
import concourse.mybir as mybir
import concourse.tile as tile
from concourse.bass import ts
from concourse.bass_utils import run_bass_kernel_spmd

B = 8
L = 1024
E = 1024
H = 8
P = 128
NT = L // P  # 8 chunks of 128
FP32 = mybir.dt.float32
F32R = mybir.dt.float32r
BF16 = mybir.dt.bfloat16
Alu = mybir.AluOpType

ATTN_OFFSET = [-3, -2, -1, 0, 0, 1, 2, 3]
KTAP = 4  # taps k in [-KTAP, KTAP]; dropped tail ~3e-6 relative
VPAD = 8  # zero pad each side of vp_T free dim (max read shift = KTAP + 3)
ULEN = L + 6  # U computed for j in [-3, L+3)

_CACHE = {}


def _build_program():
    if "nc" in _CACHE:
        return _CACHE["nc"]

    nc = bacc.Bacc("TRN2", debug=False, num_devices=B)

    ath_d = nc.dram_tensor("ath", [E, L], F32R, kind="ExternalInput")
    atb_d = nc.dram_tensor("atb", [E, L], BF16, kind="ExternalInput")
    atl_d = nc.dram_tensor("atl", [E, L], BF16, kind="ExternalInput")
    # wit variants host-pretiled as [cb, p, k, c]: per-c-block contiguous
    with_d = nc.dram_tensor("with_", [H, P, NT, P], F32R, kind="ExternalInput")
    witb_d = nc.dram_tensor("witb", [H, P, NT, P], BF16, kind="ExternalInput")
    witl_d = nc.dram_tensor("witl", [H, P, NT, P], BF16, kind="ExternalInput")
    woth_d = nc.dram_tensor("woth", [E, E], F32R, kind="ExternalInput")
    wotb_d = nc.dram_tensor("wotb", [E, E], BF16, kind="ExternalInput")
    wotl_d = nc.dram_tensor("wotl", [E, E], BF16, kind="ExternalInput")
    out_d = nc.dram_tensor("out", [L, E], FP32, kind="ExternalOutput")

    gval = [
        math.exp(-0.5 * k * k) / math.sqrt(2.0 * math.pi)
        for k in range(-KTAP, KTAP + 1)
    ]

    with tile.TileContext(nc) as tc:
        with (
            tc.tile_pool(name="glob", bufs=1) as gpool,
            tc.tile_pool(name="vpt_roll", bufs=3) as vptpool,
            tc.tile_pool(name="wit_roll", bufs=3) as witpool,
            tc.tile_pool(name="out_roll", bufs=2) as outpool,
            tc.tile_pool(name="big_ps", bufs=4, space="PSUM") as big_psum,
        ):
            # whole-kernel residents
            ath_sb = None  # declared in the scoped pool below
            woth_sb = gpool.tile([P, H, E], F32R, name="woth_sb")
            u = [gpool.tile([P, ULEN], FP32, name=f"u{c}") for c in range(H)]
            uh = [gpool.tile([P, ULEN], F32R, name=f"uh{c}") for c in range(H)]

            with tc.tile_pool(name="atpool", bufs=1) as apool:
                HK = NT // 2
                ath_sb = [apool.tile([P, HK, L], F32R, name=f"ath{i}") for i in range(2)]
                atb_sb = [apool.tile([P, HK, L], BF16, name=f"atb{i}") for i in range(2)]
                atl_sb = [apool.tile([P, HK, L], BF16, name=f"atl{i}") for i in range(2)]
                # prefetch the first two c-blocks' weights ahead of the at bulk
                wpre = {}
                for cb in range(2):
                    wh = witpool.tile([P, NT, P], F32R, name="wh")
                    wb = witpool.tile([P, NT, P], BF16, name="wb")
                    wl = witpool.tile([P, NT, P], BF16, name="wl")
                    nc.sync.dma_start(wh[:], with_d[cb])
                    nc.sync.dma_start(wb[:], witb_d[cb])
                    nc.sync.dma_start(wl[:], witl_d[cb])
                    wpre[cb] = (wh, wb, wl)
                for k in range(NT):
                    nc.sync.dma_start(ath_sb[k // HK][:, k % HK, :], ath_d[ts(k, P), :])
                for k in range(NT):
                    nc.sync.dma_start(atl_sb[k // HK][:, k % HK, :], atl_d[ts(k, P), :])
                for k in range(NT):
                    nc.sync.dma_start(atb_sb[k // HK][:, k % HK, :], atb_d[ts(k, P), :])
                for k in range(NT):
                    nc.sync.dma_start(woth_sb[:, k, :], woth_d[ts(k, P), :])

                # mm1 + conv taps, pipelined per c-block (= head)
                for cb in range(H):
                    if cb in wpre:
                        wh, wb, wl = wpre[cb]
                    else:
                        wh = witpool.tile([P, NT, P], F32R, name="wh")
                        wb = witpool.tile([P, NT, P], BF16, name="wb")
                        wl = witpool.tile([P, NT, P], BF16, name="wl")
                        nc.sync.dma_start(wh[:], with_d[cb])
                        nc.sync.dma_start(wb[:], witb_d[cb])
                        nc.sync.dma_start(wl[:], witl_d[cb])

                    vp_ps = big_psum.tile([P, E], FP32, name="big_ps", tag="big")
                    for k in range(NT):
                        nc.tensor.matmul(
                            vp_ps[:, 0:512], wh[:, k, :], ath_sb[k // HK][:, k % HK, 0:512],
                            start=(k == 0), stop=False,
                        )
                        nc.tensor.matmul(
                            vp_ps[:, 512:E], wh[:, k, :], ath_sb[k // HK][:, k % HK, 512:E],
                            start=(k == 0), stop=False,
                        )
                    for k in range(NT):
                        nc.tensor.matmul(
                            vp_ps[:, 0:512], wb[:, k, :], atl_sb[k // HK][:, k % HK, 0:512],
                            start=False, stop=False,
                        )
                        nc.tensor.matmul(
                            vp_ps[:, 512:E], wb[:, k, :], atl_sb[k // HK][:, k % HK, 512:E],
                            start=False, stop=False,
                        )
                    for k in range(NT):
                        nc.tensor.matmul(
                            vp_ps[:, 0:512], wl[:, k, :], atb_sb[k // HK][:, k % HK, 0:512],
                            start=False, stop=(k == NT - 1),
                        )
                        nc.tensor.matmul(
                            vp_ps[:, 512:E], wl[:, k, :], atb_sb[k // HK][:, k % HK, 512:E],
                            start=False, stop=(k == NT - 1),
                        )

                    vpt = vptpool.tile([P, L + 2 * VPAD], FP32, name="vpt")
                    nc.gpsimd.memset(vpt[:, 0:VPAD], 0.0)
                    nc.gpsimd.memset(vpt[:, VPAD + L : L + 2 * VPAD], 0.0)
                    nc.scalar.copy(vpt[:, VPAD : VPAD + 512], vp_ps[:, 0:512])
                    nc.scalar.copy(vpt[:, VPAD + 512 : VPAD + L], vp_ps[:, 512:E])

                    # conv taps: U[j] = sum_k g(k) vp[j+k], j in [-3, L+3)
                    # split across VectorE (7 taps) and GpSimd (4 taps) with a
                    # merge, so neither engine gates the matmul pipeline
                    base = VPAD - 3 - KTAP
                    nc.vector.tensor_scalar_mul(
                        u[cb][:], vpt[:, base : base + ULEN], gval[0]
                    )
                    for ki in range(1, 2 * KTAP + 1):
                        nc.vector.scalar_tensor_tensor(
                            u[cb][:],
                            vpt[:, base + ki : base + ki + ULEN],
                            gval[ki],
                            u[cb][:],
                            Alu.mult,
                            Alu.add,
                        )
                    nc.vector.tensor_copy(uh[cb][:], u[cb][:])

            # at-complex released; late wot parts + u low splits reuse its space
            with tc.tile_pool(name="upool", bufs=1) as upool:
                wotb_sb = upool.tile([P, H, E], BF16, name="wotb_sb")
                wotl_sb = upool.tile([P, H, E], BF16, name="wotl_sb")
                ub = [upool.tile([P, ULEN], BF16, name=f"ub{c}") for c in range(H)]
                ul = [upool.tile([P, ULEN], BF16, name=f"ul{c}") for c in range(H)]
                for k in range(NT):
                    nc.sync.dma_start(wotl_sb[:, k, :], wotl_d[ts(k, P), :])
                for cb in range(H):
                    nc.scalar.copy(ub[cb][:], u[cb][:])
                    nc.vector.tensor_tensor(
                        ul[cb][:], u[cb][:], uh[cb][:], Alu.subtract
                    )
                for k in range(NT):
                    nc.sync.dma_start(wotb_sb[:, k, :], wotb_d[ts(k, P), :])

                # mm2: out[q-block t] = U_shifted.T @ WoT; t-pairs interleaved
                # per pass so the PE has runnable work while u splits complete
                for tp in range(NT // 2):
                    pair = (2 * tp, 2 * tp + 1)
                    pps = {}
                    for t in pair:
                        pps[t] = big_psum.tile([P, E], FP32, name="big_ps", tag="big")
                    for t in pair:
                        out_ps = pps[t]
                        offs = [3 + ATTN_OFFSET[h] + t * P for h in range(H)]
                        for h in range(H):
                            nc.tensor.matmul(
                                out_ps[:, 0:512], uh[h][:, offs[h] : offs[h] + P],
                                woth_sb[:, h, 0:512], start=(h == 0), stop=False,
                            )
                            nc.tensor.matmul(
                                out_ps[:, 512:E], uh[h][:, offs[h] : offs[h] + P],
                                woth_sb[:, h, 512:E], start=(h == 0), stop=False,
                            )
                    for t in pair:
                        out_ps = pps[t]
                        offs = [3 + ATTN_OFFSET[h] + t * P for h in range(H)]
                        for h in range(H):
                            nc.tensor.matmul(
                                out_ps[:, 0:512], ub[h][:, offs[h] : offs[h] + P],
                                wotl_sb[:, h, 0:512], start=False, stop=False,
                            )
                            nc.tensor.matmul(
                                out_ps[:, 512:E], ub[h][:, offs[h] : offs[h] + P],
                                wotl_sb[:, h, 512:E], start=False, stop=False,
                            )
                    for t in pair:
                        out_ps = pps[t]
                        offs = [3 + ATTN_OFFSET[h] + t * P for h in range(H)]
                        for h in range(H):
                            nc.tensor.matmul(
                                out_ps[:, 0:512], ul[h][:, offs[h] : offs[h] + P],
                                wotb_sb[:, h, 0:512], start=False, stop=(h == H - 1),
                            )
                            nc.tensor.matmul(
                                out_ps[:, 512:E], ul[h][:, offs[h] : offs[h] + P],
                                wotb_sb[:, h, 512:E], start=False, stop=(h == H - 1),
                            )
                    for t in pair:
                        out_ps = pps[t]
                        for half in range(2):
                            out_t = outpool.tile([P, 512], FP32, name="out_t")
                            nc.scalar.copy(out_t[:], out_ps[:, ts(half, 512)])
                            nc.sync.dma_start(
                                out_d[ts(t, P), ts(half, 512)], out_t[:]
                            )

    nc.compile()
    _CACHE["nc"] = nc
    return nc


def _rne11(x):
    """Round fp32 to 11 mantissa bits (round-half-away); exact under the
    PE's float32r read (round-to-nearest at 11 bits keeps <=11-bit values)."""
    b = x.view(np.uint32)
    hi = ((b + np.uint32(0x800)) & np.uint32(0xFFFFF000)).view(np.float32)
    return hi


def _split3(x):
    x = np.ascontiguousarray(x, np.float32)
    hi = _rne11(x)
    lo = (x - hi).astype(np.float32)
    return hi, x.astype(ml_dtypes.bfloat16), lo.astype(ml_dtypes.bfloat16)


def _make_in_maps(values, key_mask, input_weights, output_weight):
    wit = np.asarray(input_weights, np.float32).T  # [e, c]
    # pre-tile wit as [cb, p, k, c]: element [cb,p,k,c] = wit[128k+p, 128cb+c]
    wit4 = np.ascontiguousarray(
        wit.reshape(NT, P, H, P).transpose(2, 1, 0, 3)
    )  # [cb, p, k, c] -> wait: see below
    woth, wotb, wotl = _split3(np.asarray(output_weight, np.float32).T)
    with_, witb, witl = _split3(wit4)
    keep = (~np.asarray(key_mask, dtype=bool)).astype(np.float32)
    in_maps = []
    for b in range(B):
        at = (np.asarray(values[b], np.float32) * keep[b][:, None]).T
        ath, atb, atl = _split3(at)
        in_maps.append(
            {
                "ath": ath, "atb": atb, "atl": atl,
                "with_": with_, "witb": witb, "witl": witl,
                "woth": woth, "wotb": wotb, "wotl": wotl,
            }
        )
    return in_maps


def _run(values, key_mask, input_weights, output_weight, trace=False):
    nc = _build_program()
    in_maps = _make_in_maps(values, key_mask, input_weights, output_weight)
    res = run_bass_kernel_spmd(nc, in_maps, core_ids=list(range(B)), trace=trace)
    out = np.stack([np.asarray(res.results[b]["out"]) for b in range(B)], axis=0)
    return out.astype(np.float32, copy=False), res


def kernel(values, queries, key_mask, input_weights, output_weight):
    out, _ = _run(values, key_mask, input_weights, output_weight, trace=False)
    return out


# revision 23
# speedup vs baseline: 1.2210x; 1.2210x over previous
"""Trainium2 Bass kernel for nn_NewAttention (sparse gaussian attention).

Reference computation (B=8, L=1024, E=1024, H=8, d=128):
    v    = (values @ Wi.T)                        # [B, L, E] per-position projection
    v    = where(key_mask, 0, v)                  # zero masked key rows
    att  = einsum('hqv,bhvd->bhqd', gauss, v_h)   # per-head gaussian positional conv
    out  = att_merged @ Wo.T                      # [B, L, E]

The gaussian weights w[h,q,v] = pdf(v - ofs_h - q; std=1) form a narrow band,
so the attention is a per-head 1-D convolution along the sequence; all heads
share the same centered gaussian, the per-head integer offset only shifts
where the result is read.  Per core (data-parallel over B, one batch per
NeuronCore, no collectives):

  mm1:  vp_T[c, v] = WiT[e, c].T @ AT[e, v]     (PE; AT = values[b].T host-prepped)
  conv: U[c, j] = sum_k g(k) * vp_T[c, j + k], j in [-3, 1027), k in [-5, 5]
        -- 11 fused multiply-adds per head block on the otherwise-idle VectorE.
  mm2:  out[q, m] = U[c, q + ofs_h].T @ WoT[c, m] (PE; per-head shift is just a
        free-dim offset into U).

Matmul precision: the PE's native fp32 mode costs 4 cycles/column (2 LOW_HIGH
passes).  Instead each product X @ W is computed in 3 single-cycle passes:
    X_hi @ W_hi  (float32r: fp32 bits rounded to 11 mantissa bits on read)
  + bf16(X)  @ bf16(W - W_hi)
  + bf16(X - X_hi) @ bf16(W)
where X_hi = round-to-nearest-11-bits(X) so the f32r read is exact.  The hi*hi
products are exact in fp32 accumulation; cross terms are ~2^-12 corrections
with ~2^-8 relative error of their own -> total ~fp32 accuracy at ~3/4 the PE
cost.  Weight splits are host-side; the U split runs on VectorE/ScalarE.
PSUM accumulates everything in fp32.
"""

import math

import numpy as np
import ml_dtypes

import concourse.bacc as bacc
import concourse.mybir as mybir
import concourse.tile as tile
from concourse.bass import ts
from concourse.bass_utils import run_bass_kernel_spmd

B = 8
L = 1024
E = 1024
H = 8
P = 128
NT = L // P  # 8 chunks of 128
FP32 = mybir.dt.float32
F32R = mybir.dt.float32r
BF16 = mybir.dt.bfloat16
Alu = mybir.AluOpType

ATTN_OFFSET = [-3, -2, -1, 0, 0, 1, 2, 3]
KTAP = 4  # taps k in [-KTAP, KTAP]; dropped tail ~3e-6 relative
VPAD = 8  # zero pad each side of vp_T free dim (max read shift = KTAP + 3)
ULEN = L + 6  # U computed for j in [-3, L+3)

_CACHE = {}


def _build_program():
    if "nc" in _CACHE:
        return _CACHE["nc"]

    nc = bacc.Bacc("TRN2", debug=False, num_devices=B)

    ath_d = nc.dram_tensor("ath", [E, L], F32R, kind="ExternalInput")
    atb_d = nc.dram_tensor("atb", [E, L], BF16, kind="ExternalInput")
    atl_d = nc.dram_tensor("atl", [E, L], BF16, kind="ExternalInput")
    # wit variants host-pretiled as [cb, p, k, c]: per-c-block contiguous
    with_d = nc.dram_tensor("with_", [H, P, NT, P], F32R, kind="ExternalInput")
    witb_d = nc.dram_tensor("witb", [H, P, NT, P], BF16, kind="ExternalInput")
    witl_d = nc.dram_tensor("witl", [H, P, NT, P], BF16, kind="ExternalInput")
    woth_d = nc.dram_tensor("woth", [E, E], F32R, kind="ExternalInput")
    wotb_d = nc.dram_tensor("wotb", [E, E], BF16, kind="ExternalInput")
    wotl_d = nc.dram_tensor("wotl", [E, E], BF16, kind="ExternalInput")
    out_d = nc.dram_tensor("out", [L, E], FP32, kind="ExternalOutput")

    gval = [
        math.exp(-0.5 * k * k) / math.sqrt(2.0 * math.pi)
        for k in range(-KTAP, KTAP + 1)
    ]

    with tile.TileContext(nc) as tc:
        with (
            tc.tile_pool(name="glob", bufs=1) as gpool,
            tc.tile_pool(name="vpt_roll", bufs=3) as vptpool,
            tc.tile_pool(name="wit_roll", bufs=3) as witpool,
            tc.tile_pool(name="out_roll", bufs=2) as outpool,
            tc.tile_pool(name="big_ps", bufs=4, space="PSUM") as big_psum,
        ):
            # whole-kernel residents
            ath_sb = None  # declared in the scoped pool below
            woth_sb = gpool.tile([P, H, E], F32R, name="woth_sb")
            wotb_sb = gpool.tile([P, H, E], BF16, name="wotb_sb")
            wotl_sb = gpool.tile([P, H, E], BF16, name="wotl_sb")
            u = [gpool.tile([P, ULEN], FP32, name=f"u{c}") for c in range(H)]

            with tc.tile_pool(name="atpool", bufs=1) as apool:
                HK = NT // 2
                ath_sb = [apool.tile([P, HK, L], F32R, name=f"ath{i}") for i in range(2)]
                atb_sb = [apool.tile([P, HK, L], BF16, name=f"atb{i}") for i in range(2)]
                atl_sb = [apool.tile([P, HK, L], BF16, name=f"atl{i}") for i in range(2)]
                # prefetch the first two c-blocks' weights ahead of the at bulk
                wpre = {}
                for cb in range(2):
                    wh = witpool.tile([P, NT, P], F32R, name="wh")
                    wb = witpool.tile([P, NT, P], BF16, name="wb")
                    wl = witpool.tile([P, NT, P], BF16, name="wl")
                    nc.sync.dma_start(wh[:], with_d[cb])
                    nc.sync.dma_start(wb[:], witb_d[cb])
                    nc.sync.dma_start(wl[:], witl_d[cb])
                    wpre[cb] = (wh, wb, wl)
                for k in range(NT):
                    nc.sync.dma_start(ath_sb[k // HK][:, k % HK, :], ath_d[ts(k, P), :])
                for k in range(NT):
                    nc.sync.dma_start(atl_sb[k // HK][:, k % HK, :], atl_d[ts(k, P), :])
                for k in range(NT):
                    nc.sync.dma_start(atb_sb[k // HK][:, k % HK, :], atb_d[ts(k, P), :])

                # mm1 + conv taps, pipelined per c-block (= head)
                for cb in range(H):
                    if cb in wpre:
                        wh, wb, wl = wpre[cb]
                    else:
                        wh = witpool.tile([P, NT, P], F32R, name="wh")
                        wb = witpool.tile([P, NT, P], BF16, name="wb")
                        wl = witpool.tile([P, NT, P], BF16, name="wl")
                        nc.sync.dma_start(wh[:], with_d[cb])
                        nc.sync.dma_start(wb[:], witb_d[cb])
                        nc.sync.dma_start(wl[:], witl_d[cb])

                    vp_ps = big_psum.tile([P, E], FP32, name="big_ps", tag="big")
                    for k in range(NT):
                        nc.tensor.matmul(
                            vp_ps[:, 0:512], wh[:, k, :], ath_sb[k // HK][:, k % HK, 0:512],
                            start=(k == 0), stop=False,
                        )
                        nc.tensor.matmul(
                            vp_ps[:, 512:E], wh[:, k, :], ath_sb[k // HK][:, k % HK, 512:E],
                            start=(k == 0), stop=False,
                        )
                    for k in range(NT):
                        nc.tensor.matmul(
                            vp_ps[:, 0:512], wb[:, k, :], atl_sb[k // HK][:, k % HK, 0:512],
                            start=False, stop=False,
                        )
                        nc.tensor.matmul(
                            vp_ps[:, 512:E], wb[:, k, :], atl_sb[k // HK][:, k % HK, 512:E],
                            start=False, stop=False,
                        )
                    for k in range(NT):
                        nc.tensor.matmul(
                            vp_ps[:, 0:512], wl[:, k, :], atb_sb[k // HK][:, k % HK, 0:512],
                            start=False, stop=(k == NT - 1),
                        )
                        nc.tensor.matmul(
                            vp_ps[:, 512:E], wl[:, k, :], atb_sb[k // HK][:, k % HK, 512:E],
                            start=False, stop=(k == NT - 1),
                        )

                    vpt = vptpool.tile([P, L + 2 * VPAD], FP32, name="vpt")
                    nc.gpsimd.memset(vpt[:, 0:VPAD], 0.0)
                    nc.gpsimd.memset(vpt[:, VPAD + L : L + 2 * VPAD], 0.0)
                    nc.scalar.copy(vpt[:, VPAD : VPAD + 512], vp_ps[:, 0:512])
                    nc.scalar.copy(vpt[:, VPAD + 512 : VPAD + L], vp_ps[:, 512:E])

                    # conv taps: U[j] = sum_k g(k) vp[j+k], j in [-3, L+3)
                    # split across VectorE (7 taps) and GpSimd (4 taps) with a
                    # merge, so neither engine gates the matmul pipeline
                    base = VPAD - 3 - KTAP
                    nc.vector.tensor_scalar_mul(
                        u[cb][:], vpt[:, base : base + ULEN], gval[0]
                    )
                    for ki in range(1, 2 * KTAP + 1):
                        nc.vector.scalar_tensor_tensor(
                            u[cb][:],
                            vpt[:, base + ki : base + ki + ULEN],
                            gval[ki],
                            u[cb][:],
                            Alu.mult,
                            Alu.add,
                        )

            for k in range(NT):
                nc.sync.dma_start(woth_sb[:, k, :], woth_d[ts(k, P), :])
                nc.sync.dma_start(wotb_sb[:, k, :], wotb_d[ts(k, P), :])
                nc.sync.dma_start(wotl_sb[:, k, :], wotl_d[ts(k, P), :])

            # at-complex released; u splits reuse its space (post-mm1)
            with tc.tile_pool(name="upool", bufs=1) as upool:
                uh = [upool.tile([P, ULEN], F32R, name=f"uh{c}") for c in range(H)]
                ub = [upool.tile([P, ULEN], BF16, name=f"ub{c}") for c in range(H)]
                ul = [upool.tile([P, ULEN], BF16, name=f"ul{c}") for c in range(H)]
                for cb in range(H):
                    nc.vector.tensor_copy(uh[cb][:], u[cb][:])
                    nc.scalar.copy(ub[cb][:], u[cb][:])
                for cb in range(H):
                    nc.vector.tensor_tensor(
                        ul[cb][:], u[cb][:], uh[cb][:], Alu.subtract
                    )

                # mm2: out[q-block t] = U_shifted.T @ WoT; t-pairs interleaved
                # per pass so the PE has runnable work while u splits complete
                for tp in range(NT // 2):
                    pair = (2 * tp, 2 * tp + 1)
                    pps = {}
                    for t in pair:
                        pps[t] = big_psum.tile([P, E], FP32, name="big_ps", tag="big")
                    for t in pair:
                        out_ps = pps[t]
                        offs = [3 + ATTN_OFFSET[h] + t * P for h in range(H)]
                        for h in range(H):
                            nc.tensor.matmul(
                                out_ps[:, 0:512], uh[h][:, offs[h] : offs[h] + P],
                                woth_sb[:, h, 0:512], start=(h == 0), stop=False,
                            )
                            nc.tensor.matmul(
                                out_ps[:, 512:E], uh[h][:, offs[h] : offs[h] + P],
                                woth_sb[:, h, 512:E], start=(h == 0), stop=False,
                            )
                    for t in pair:
                        out_ps = pps[t]
                        offs = [3 + ATTN_OFFSET[h] + t * P for h in range(H)]
                        for h in range(H):
                            nc.tensor.matmul(
                                out_ps[:, 0:512], ub[h][:, offs[h] : offs[h] + P],
                                wotl_sb[:, h, 0:512], start=False, stop=False,
                            )
                            nc.tensor.matmul(
                                out_ps[:, 512:E], ub[h][:, offs[h] : offs[h] + P],
                                wotl_sb[:, h, 512:E], start=False, stop=False,
                            )
                    for t in pair:
                        out_ps = pps[t]
                        offs = [3 + ATTN_OFFSET[h] + t * P for h in range(H)]
                        for h in range(H):
                            nc.tensor.matmul(
                                out_ps[:, 0:512], ul[h][:, offs[h] : offs[h] + P],
                                wotb_sb[:, h, 0:512], start=False, stop=(h == H - 1),
                            )
                            nc.tensor.matmul(
                                out_ps[:, 512:E], ul[h][:, offs[h] : offs[h] + P],
                                wotb_sb[:, h, 512:E], start=False, stop=(h == H - 1),
                            )
                    for t in pair:
                        out_ps = pps[t]
                        for half in range(2):
                            out_t = outpool.tile([P, 512], FP32, name="out_t")
                            nc.scalar.copy(out_t[:], out_ps[:, ts(half, 512)])
                            nc.sync.dma_start(
                                out_d[ts(t, P), ts(half, 512)], out_t[:]
                            )

    nc.compile()
    _CACHE["nc"] = nc
    return nc


def _rne11(x):
    """Round fp32 to 11 mantissa bits (round-half-away); exact under the
    PE's float32r read (round-to-nearest at 11 bits keeps <=11-bit values)."""
    b = x.view(np.uint32)
    hi = ((b + np.uint32(0x800)) & np.uint32(0xFFFFF000)).view(np.float32)
    return hi


def _split3(x):
    x = np.ascontiguousarray(x, np.float32)
    hi = _rne11(x)
    lo = (x - hi).astype(np.float32)
    return hi, x.astype(ml_dtypes.bfloat16), lo.astype(ml_dtypes.bfloat16)


def _make_in_maps(values, key_mask, input_weights, output_weight):
    wit = np.asarray(input_weights, np.float32).T  # [e, c]
    # pre-tile wit as [cb, p, k, c]: element [cb,p,k,c] = wit[128k+p, 128cb+c]
    wit4 = np.ascontiguousarray(
        wit.reshape(NT, P, H, P).transpose(2, 1, 0, 3)
    )  # [cb, p, k, c] -> wait: see below
    woth, wotb, wotl = _split3(np.asarray(output_weight, np.float32).T)
    with_, witb, witl = _split3(wit4)
    keep = (~np.asarray(key_mask, dtype=bool)).astype(np.float32)
    in_maps = []
    for b in range(B):
        at = (np.asarray(values[b], np.float32) * keep[b][:, None]).T
        ath, atb, atl = _split3(at)
        in_maps.append(
            {
                "ath": ath, "atb": atb, "atl": atl,
                "with_": with_, "witb": witb, "witl": witl,
                "woth": woth, "wotb": wotb, "wotl": wotl,
            }
        )
    return in_maps


def _run(values, key_mask, input_weights, output_weight, trace=False):
    nc = _build_program()
    in_maps = _make_in_maps(values, key_mask, input_weights, output_weight)
    res = run_bass_kernel_spmd(nc, in_maps, core_ids=list(range(B)), trace=trace)
    out = np.stack([np.asarray(res.results[b]["out"]) for b in range(B)], axis=0)
    return out.astype(np.float32, copy=False), res


def kernel(values, queries, key_mask, input_weights, output_weight):
    out, _ = _run(values, key_mask, input_weights, output_weight, trace=False)
    return out
